# revision 20
# baseline (speedup 1.0000x reference)
"""Grouped-expert FFN (MoE) Trainium2 kernel.

Problem: E=64 experts, each x[1024,512] @ w1[512,2048] -> +b1 -> gelu(erf)
-> @ w2[2048,512] -> +b2, rows >= valid_load[e] zeroed.

Strategy (v2 — unit-based, bf16):
 - Work is decomposed into column "units": every core runs the same static
   sequence of unit widths (SPMD), but the host assigns ANY (expert,
   column-range) piece to each (core, unit) cell, with a per-unit copy of
   that expert's weights in DRAM. This removes the per-slot max-over-cores
   padding of expert-parallel layouts: ~4800 columns/core vs 5472.
 - The unit width multiset is optimized at runtime by a deterministic
   annealer over per-expert cuts (rank-deal dominance: pieces sorted desc,
   unit j = max of piece ranks [8j, 8j+8)).
 - All matmul operands are bf16 (PE streams bf16 at 1 elem/cell/cycle,
   identical peak to fp32r, but half the HBM traffic; PSUM accumulates
   fp32). rel err ~3e-3 vs the 2e-2 gate.
 - Host transposes x per expert (xT [D,C]) so the device contracts over D
   with no on-chip transposes; both biases land on the partition axis ->
   free via ACT activation bias. GEMM1: hT = w1-tiles.T @ xT, GEMM2:
   yT = w2-tiles.T @ hT.
 - Unit 0's w1 is DMA'd in 16 m-slices so the first matmul starts ~2us
   after queue init instead of waiting for the full 2MB tile; y is written
   back per m-tile to shrink the kernel tail.
"""

import random

import numpy as np

import concourse.bass as bass
import concourse.bacc as bacc
import concourse.tile as tile
from concourse import mybir
from concourse.bass_utils import run_bass_kernel_spmd

E, CAP, D, H = 64, 1024, 512, 2048
N_CORES = 8
KT1, MT1 = D // 128, H // 128     # GEMM1: 4 contraction tiles, 16 out tiles
KT2, MT2 = H // 128, D // 128     # GEMM2: 16 contraction tiles, 4 out tiles
WMAX = 512                        # PSUM bank = 512 fp32 columns

F32 = mybir.dt.float32
BF16 = mybir.dt.bfloat16

_PROGRAM_CACHE: dict[tuple, object] = {}
_SCHEDULE_CACHE: dict[tuple, tuple] = {}
LAST_RESULT = None               # test harness introspection


# ----------------------------- scheduling -----------------------------
# Work = "columns" (units): each unit j has width U_j and 8 slots (one per
# core). An expert covers its v_e columns with a set of slots; coverage of a
# slot = its unit's width. Minimize sum(U_j) (= PE columns per core) by
# slot-level annealing with a width-tightening post-pass.

_Q = 16                           # width quantum


def _slot_init(v):
    """Deal-based start: equal cuts per expert, rank-dealt into columns."""
    pieces = []
    for e, ve in enumerate(v):
        ve = int(ve)
        if ve <= 0:
            continue
        k = -(-ve // WMAX)
        w = max(_Q, -(-(-(-ve // k)) // _Q) * _Q)
        ws = [w] * k
        over = sum(ws) - ve
        t = over // _Q * _Q
        i = len(ws) - 1
        while t > 0 and i >= 0:
            d = min(t, ws[i] - _Q)
            ws[i] -= d
            t -= d
            i -= 1
        for w_ in ws:
            pieces.append((w_, e))
    pieces.sort(key=lambda p: (-p[0], p[1]))
    cols = []
    for j in range(0, len(pieces), N_CORES):
        grp = pieces[j:j + N_CORES]
        cols.append([grp[0][0], [e for _, e in grp]])
    return cols


def _covers(cols, n):
    cov = [0] * n
    for w, slots in cols:
        for e in slots:
            cov[e] += w
    return cov


def _tighten(cols, v):
    cov = _covers(cols, len(v))
    changed = True
    while changed:
        changed = False
        for c in cols:
            w, slots = c
            if not slots:
                continue
            cnt = {}
            for e in slots:
                cnt[e] = cnt.get(e, 0) + 1
            d = min((cov[e] - v[e]) // m for e, m in cnt.items())
            d = min(d // _Q * _Q, w - _Q)
            if d > 0:
                c[0] -= d
                for e in slots:
                    cov[e] -= d
                changed = True
    cols[:] = [c for c in cols if c[1]]
    return cols


def _slot_anneal(v, iters=60000):
    import copy
    rng = random.Random(11)
    cur = _slot_init(v)
    _tighten(cur, v)
    cur_c = sum(c[0] for c in cur)
    best = copy.deepcopy(cur)
    best_c = cur_c
    temp = 300.0
    for _ in range(iters):
        temp = max(3.0, temp * 0.99995)
        cand = copy.deepcopy(cur)
        cov = _covers(cand, len(v))
        op = rng.random()
        ok = False
        if op < 0.4:
            j = rng.randrange(len(cand))
            if cand[j][1]:
                si = rng.randrange(len(cand[j][1]))
                e = cand[j][1][si]
                j2 = rng.randrange(len(cand))
                if j2 != j and len(cand[j2][1]) < N_CORES:
                    if cov[e] - cand[j][0] + cand[j2][0] >= v[e]:
                        cand[j][1].pop(si)
                        cand[j2][1].append(e)
                        ok = True
        elif op < 0.7:
            j = rng.randrange(len(cand))
            j2 = rng.randrange(len(cand))
            if j != j2 and cand[j][1] and cand[j2][1]:
                a = rng.randrange(len(cand[j][1]))
                b = rng.randrange(len(cand[j2][1]))
                e1 = cand[j][1][a]
                e2 = cand[j2][1][b]
                if e1 != e2:
                    dd = cand[j][0] - cand[j2][0]
                    if cov[e1] - dd >= v[e1] and cov[e2] + dd >= v[e2]:
                        cand[j][1][a] = e2
                        cand[j2][1][b] = e1
                        ok = True
        elif op < 0.85:
            j = rng.randrange(len(cand))
            if cand[j][1]:
                si = rng.randrange(len(cand[j][1]))
                e = cand[j][1][si]
                if cov[e] - cand[j][0] >= v[e]:
                    cand[j][1].pop(si)
                    ok = True
        else:
            e = rng.randrange(len(v))
            if v[e] > 0 and cand:
                j = rng.randrange(len(cand))
                if len(cand[j][1]) < N_CORES:
                    cand[j][1].append(e)
                    ok = True
        if not ok:
            continue
        _tighten(cand, v)
        new_c = sum(c[0] for c in cand)
        if new_c <= cur_c or rng.random() < np.exp((cur_c - new_c) / temp):
            cur = cand
            cur_c = new_c
            if new_c < best_c:
                best_c = new_c
                best = copy.deepcopy(cand)
    return best


def _schedule(v_key):
    """v (tuple of 64 ints) -> (unit widths desc, assign[j][c] = (e, c0, cov)
    or None). All cores share the width sequence."""
    if v_key in _SCHEDULE_CACHE:
        return _SCHEDULE_CACHE[v_key]
    v = [int(x) for x in v_key]
    cols = _slot_anneal(v)
    cols.sort(key=lambda c: -c[0])
    widths = tuple(c[0] for c in cols)
    nunits = len(widths)
    # expert -> list of (unit width, unit idx, core)
    slots = {}
    for j, (w, sl) in enumerate(cols):
        for c, e in enumerate(sl):
            slots.setdefault(e, []).append((w, j, c))
    assign = [[None] * N_CORES for _ in range(nunits)]
    for e, sl in slots.items():
        sl.sort(key=lambda s: -s[0])
        cum = 0
        for wu, j, c in sl:
            if cum >= v[e]:
                continue  # surplus slot -> dummy
            c0 = min(cum, CAP - wu)
            end = min(c0 + wu, v[e])
            assign[j][c] = (e, c0, end - c0)
            cum = end
        assert cum >= v[e], (e, v[e], sl)
    _SCHEDULE_CACHE[v_key] = (widths, assign)
    return widths, assign


# ----------------------------- device program -----------------------------
# All DRAM operands are pre-packed on the host into partition-major layouts
# so every DMA descriptor is 128 contiguous lines: trigger instructions cost
# engine time proportional to line count (HW-measured ~0.7us per 512-line
# trigger, 7.6us for a 2048-line w2) and a slow trigger on the ACT engine
# delays the activations that recycle PSUM banks.

def _build_program(widths: tuple):
    nc = bacc.Bacc(None, target_bir_lowering=False)
    K = len(widths)

    xt = nc.dram_tensor("xt", [K, 128, KT1 * WMAX], BF16, kind="ExternalInput")
    w1g = nc.dram_tensor("w1g", [K, 128, KT1, H], BF16, kind="ExternalInput")
    w2g = nc.dram_tensor("w2g", [K, 128, KT2, D], BF16, kind="ExternalInput")
    bg = nc.dram_tensor("bg", [K, 128, MT1 + MT2], F32, kind="ExternalInput")
    yt = nc.dram_tensor("yt", [K, 128, MT2 * WMAX], F32, kind="ExternalOutput")
    scr = nc.dram_tensor("scr", [128, 64], F32, kind="ExternalOutput")

    Gelu = mybir.ActivationFunctionType.Gelu
    Ident = mybir.ActivationFunctionType.Identity

    # 2nd-smallest unit first: its x arrives fast AND its ~14us of compute
    # covers the constant ~4.2MB/unit weight-DMA startup debt (a smaller
    # first unit starves the PE and re-throttles HAM); smallest unit last so
    # the final y writeback drain is short
    if K >= 3:
        emit_order = [K - 2] + list(range(K - 2)) + [K - 1]
    else:
        emit_order = list(range(K - 1, -1, -1))

    with tile.TileContext(nc) as tc:
        with (
            tc.tile_pool(name="wu", bufs=1) as wu,
            tc.tile_pool(name="w1p", bufs=2) as w1p,
            tc.tile_pool(name="w2p", bufs=2) as w2p,
            tc.tile_pool(name="bp", bufs=2) as bp,
            tc.tile_pool(name="xp", bufs=3) as xp,
            tc.tile_pool(name="hp", bufs=2) as hp,
            tc.tile_pool(name="yp", bufs=2) as yp,
            tc.tile_pool(name="ps_h", bufs=4, space="PSUM") as ps_h,
            tc.tile_pool(name="ps_y", bufs=4, space="PSUM") as ps_y,
        ):
            # PE pre-warm: ~5us of dummy matmuls during the initial DMA wait
            # flips the HAM clock gate to 8/8 before real work arrives
            warm = wu.tile([128, 64], BF16, tag="warm")
            nc.gpsimd.memset(warm, 0.0)
            wps = None
            for _ in range(288):
                wps = ps_h.tile([128, 64], F32, tag="psh")
                nc.tensor.matmul(
                    wps[:64, :], lhsT=warm, rhs=warm, start=True, stop=True)
            wout = wu.tile([128, 64], F32, tag="wout")
            nc.scalar.activation(wout[:64, :], wps[:64, :], Ident)
            nc.gpsimd.dma_start(out=scr[:64, :], in_=wout[:64, :])

            for ei, j in enumerate(emit_order):
                W = widths[j]
                w1_t = w1p.tile([128, KT1, H], BF16, tag="w1")
                x_t = xp.tile([128, KT1 * WMAX], BF16, tag="x")
                nc.sync.dma_start(out=x_t[:, :KT1 * W], in_=xt[j][:, :KT1 * W])
                if ei < 2:
                    # k-sliced early loads: GEMM1 starts after slice 0 + x
                    # (packed layout makes extra triggers cheap: ~0.2us each)
                    for k in range(KT1):
                        nc.sync.dma_start(
                            out=w1_t[:, k, :], in_=w1g[j][:, k, :])
                else:
                    nc.sync.dma_start(out=w1_t, in_=w1g[j])
                b_t = bp.tile([128, MT1 + MT2], F32, tag="b")
                nc.scalar.dma_start(out=b_t, in_=bg[j])
                # w2 rides the second HWDGE ring (ACT): needed only for GEMM2
                w2_t = w2p.tile([128, KT2, D], BF16, tag="w2")
                if ei >= 2:
                    nc.scalar.dma_start(out=w2_t, in_=w2g[j])

                h_t = hp.tile([128, KT2, WMAX], BF16, tag="h")
                for m in range(MT1):
                    ps = ps_h.tile([128, WMAX], F32, tag="psh")
                    for k in range(KT1):
                        nc.tensor.matmul(
                            ps[:, :W],
                            lhsT=w1_t[:, k, m * 128:(m + 1) * 128],
                            rhs=x_t[:, k * W:(k + 1) * W],
                            start=(k == 0),
                            stop=(k == KT1 - 1),
                        )
                    nc.scalar.activation(
                        h_t[:, m, :W], ps[:, :W], Gelu, bias=b_t[:, m:m + 1])
                if ei < 2:
                    # early units: w2 deferred past GEMM1 (frees startup HBM
                    # bandwidth for the critical w1/x path) and k-group
                    # sliced so GEMM2 unblocks progressively
                    for a in range(4):
                        nc.scalar.dma_start(
                            out=w2_t[:, 4 * a:4 * a + 4, :],
                            in_=w2g[j][:, 4 * a:4 * a + 4, :])

                y_t = yp.tile([128, MT2 * WMAX], F32, tag="y")
                last = ei == K - 1
                for dm in range(MT2):
                    ps2 = ps_y.tile([128, WMAX], F32, tag="psy")
                    for k in range(KT2):
                        nc.tensor.matmul(
                            ps2[:, :W],
                            lhsT=w2_t[:, k, dm * 128:(dm + 1) * 128],
                            rhs=h_t[:, k, :W],
                            start=(k == 0),
                            stop=(k == KT2 - 1),
                        )
                    nc.scalar.activation(
                        y_t[:, dm * W:(dm + 1) * W], ps2[:, :W], Ident,
                        bias=b_t[:, MT1 + dm:MT1 + dm + 1])
                    if last:
                        # final unit: write out per m-tile to shrink the tail
                        nc.gpsimd.dma_start(
                            out=yt[j][:, dm * W:(dm + 1) * W],
                            in_=y_t[:, dm * W:(dm + 1) * W])
                if not last:
                    nc.gpsimd.dma_start(
                        out=yt[j][:, :MT2 * W], in_=y_t[:, :MT2 * W])

    nc.compile()
    return nc


# ----------------------------- host wrapper -----------------------------

def kernel(packed_inputs, valid_load, w1, b1, w2, b2, _trace=False, **_):
    global LAST_RESULT
    packed_inputs = np.asarray(packed_inputs, np.float32)
    w1 = np.asarray(w1, np.float32)
    b1 = np.asarray(b1, np.float32)
    w2 = np.asarray(w2, np.float32)
    b2 = np.asarray(b2, np.float32)
    v = np.asarray(valid_load).astype(np.int64)

    out = np.zeros((E, CAP, D), np.float32)
    if int(v.max()) <= 0:
        return out

    widths, assign = _schedule(tuple(int(x) for x in v))
    K = len(widths)

    if widths not in _PROGRAM_CACHE:
        _PROGRAM_CACHE[widths] = _build_program(widths)
    nc = _PROGRAM_CACHE[widths]

    bf16 = mybir.dt.np(BF16)
    # xT per expert, partition-major k-tiles: [E, KT1, 128, CAP]
    xt_all = np.ascontiguousarray(
        packed_inputs.transpose(0, 2, 1).reshape(E, KT1, 128, CAP)
    ).astype(bf16)
    # w1 partition-major: [E, 128, KT1, H]; w2: [E, 128, KT2, D]
    w1r = np.ascontiguousarray(
        w1.astype(bf16).reshape(E, KT1, 128, H).transpose(0, 2, 1, 3))
    w2r = np.ascontiguousarray(
        w2.astype(bf16).reshape(E, KT2, 128, D).transpose(0, 2, 1, 3))
    b1r = b1.reshape(E, MT1, 128).transpose(0, 2, 1)     # [E, 128, MT1]
    b2r = b2.reshape(E, MT2, 128).transpose(0, 2, 1)
    bgr = np.ascontiguousarray(
        np.concatenate([b1r, b2r], axis=2), np.float32)   # [E, 128, 20]

    in_maps = []
    for c in range(N_CORES):
        xtc = np.zeros((K, 128, KT1 * WMAX), bf16)
        w1c = np.zeros((K, 128, KT1, H), bf16)
        w2c = np.zeros((K, 128, KT2, D), bf16)
        bc = np.zeros((K, 128, MT1 + MT2), np.float32)
        for j, W in enumerate(widths):
            pc = assign[j][c]
            if pc is None:
                continue
            e, c0, _cov = pc
            # [KT1, 128, W] -> [128, KT1*W]
            xs = xt_all[e][:, :, c0:c0 + W]
            xtc[j, :, :KT1 * W] = xs.transpose(1, 0, 2).reshape(128, KT1 * W)
            w1c[j] = w1r[e]
            w2c[j] = w2r[e]
            bc[j] = bgr[e]
        in_maps.append({"xt": xtc, "w1g": w1c, "w2g": w2c, "bg": bc})

    res = run_bass_kernel_spmd(nc, in_maps, list(range(N_CORES)), trace=_trace)
    LAST_RESULT = res

    for c in range(N_CORES):
        ytc = res.results[c]["yt"]
        for j, W in enumerate(widths):
            pc = assign[j][c]
            if pc is None:
                continue
            e, c0, cov = pc
            # [128, MT2*W] -> [(m p), W] = yT unit
            yu = ytc[j][:, :MT2 * W].reshape(128, MT2, W)
            yu = yu.transpose(1, 0, 2).reshape(D, W)
            out[e, c0:c0 + cov, :] = yu[:, :cov].T
    return out


# revision 22
# speedup vs baseline: 1.0103x; 1.0103x over previous
"""Grouped-expert FFN (MoE) Trainium2 kernel.

Problem: E=64 experts, each x[1024,512] @ w1[512,2048] -> +b1 -> gelu(erf)
-> @ w2[2048,512] -> +b2, rows >= valid_load[e] zeroed.

Strategy (v2 — unit-based, bf16):
 - Work is decomposed into column "units": every core runs the same static
   sequence of unit widths (SPMD), but the host assigns ANY (expert,
   column-range) piece to each (core, unit) cell, with a per-unit copy of
   that expert's weights in DRAM. This removes the per-slot max-over-cores
   padding of expert-parallel layouts: ~4800 columns/core vs 5472.
 - The unit width multiset is optimized at runtime by a deterministic
   annealer over per-expert cuts (rank-deal dominance: pieces sorted desc,
   unit j = max of piece ranks [8j, 8j+8)).
 - All matmul operands are bf16 (PE streams bf16 at 1 elem/cell/cycle,
   identical peak to fp32r, but half the HBM traffic; PSUM accumulates
   fp32). rel err ~3e-3 vs the 2e-2 gate.
 - Host transposes x per expert (xT [D,C]) so the device contracts over D
   with no on-chip transposes; both biases land on the partition axis ->
   free via ACT activation bias. GEMM1: hT = w1-tiles.T @ xT, GEMM2:
   yT = w2-tiles.T @ hT.
 - Unit 0's w1 is DMA'd in 16 m-slices so the first matmul starts ~2us
   after queue init instead of waiting for the full 2MB tile; y is written
   back per m-tile to shrink the kernel tail.
"""

import random

import numpy as np

import concourse.bass as bass
import concourse.bacc as bacc
import concourse.tile as tile
from concourse import mybir
from concourse.bass_utils import run_bass_kernel_spmd

E, CAP, D, H = 64, 1024, 512, 2048
N_CORES = 8
KT1, MT1 = D // 128, H // 128     # GEMM1: 4 contraction tiles, 16 out tiles
KT2, MT2 = H // 128, D // 128     # GEMM2: 16 contraction tiles, 4 out tiles
WMAX = 512                        # PSUM bank = 512 fp32 columns

F32 = mybir.dt.float32
BF16 = mybir.dt.bfloat16

_PROGRAM_CACHE: dict[tuple, object] = {}
_SCHEDULE_CACHE: dict[tuple, tuple] = {}
LAST_RESULT = None               # test harness introspection


# ----------------------------- scheduling -----------------------------
# Work = "columns" (units): each unit j has width U_j and 8 slots (one per
# core). An expert covers its v_e columns with a set of slots; coverage of a
# slot = its unit's width. Minimize sum(U_j) (= PE columns per core) by
# slot-level annealing with a width-tightening post-pass.

_Q = 16                           # width quantum


def _slot_init(v):
    """Deal-based start: equal cuts per expert, rank-dealt into columns."""
    pieces = []
    for e, ve in enumerate(v):
        ve = int(ve)
        if ve <= 0:
            continue
        k = -(-ve // WMAX)
        w = max(_Q, -(-(-(-ve // k)) // _Q) * _Q)
        ws = [w] * k
        over = sum(ws) - ve
        t = over // _Q * _Q
        i = len(ws) - 1
        while t > 0 and i >= 0:
            d = min(t, ws[i] - _Q)
            ws[i] -= d
            t -= d
            i -= 1
        for w_ in ws:
            pieces.append((w_, e))
    pieces.sort(key=lambda p: (-p[0], p[1]))
    cols = []
    for j in range(0, len(pieces), N_CORES):
        grp = pieces[j:j + N_CORES]
        cols.append([grp[0][0], [e for _, e in grp]])
    return cols


def _covers(cols, n):
    cov = [0] * n
    for w, slots in cols:
        for e in slots:
            cov[e] += w
    return cov


def _tighten(cols, v):
    cov = _covers(cols, len(v))
    changed = True
    while changed:
        changed = False
        for c in cols:
            w, slots = c
            if not slots:
                continue
            cnt = {}
            for e in slots:
                cnt[e] = cnt.get(e, 0) + 1
            d = min((cov[e] - v[e]) // m for e, m in cnt.items())
            d = min(d // _Q * _Q, w - _Q)
            if d > 0:
                c[0] -= d
                for e in slots:
                    cov[e] -= d
                changed = True
    cols[:] = [c for c in cols if c[1]]
    return cols


def _slot_anneal(v, iters=60000):
    import copy
    rng = random.Random(11)
    cur = _slot_init(v)
    _tighten(cur, v)
    cur_c = sum(c[0] for c in cur)
    best = copy.deepcopy(cur)
    best_c = cur_c
    temp = 300.0
    for _ in range(iters):
        temp = max(3.0, temp * 0.99995)
        cand = copy.deepcopy(cur)
        cov = _covers(cand, len(v))
        op = rng.random()
        ok = False
        if op < 0.4:
            j = rng.randrange(len(cand))
            if cand[j][1]:
                si = rng.randrange(len(cand[j][1]))
                e = cand[j][1][si]
                j2 = rng.randrange(len(cand))
                if j2 != j and len(cand[j2][1]) < N_CORES:
                    if cov[e] - cand[j][0] + cand[j2][0] >= v[e]:
                        cand[j][1].pop(si)
                        cand[j2][1].append(e)
                        ok = True
        elif op < 0.7:
            j = rng.randrange(len(cand))
            j2 = rng.randrange(len(cand))
            if j != j2 and cand[j][1] and cand[j2][1]:
                a = rng.randrange(len(cand[j][1]))
                b = rng.randrange(len(cand[j2][1]))
                e1 = cand[j][1][a]
                e2 = cand[j2][1][b]
                if e1 != e2:
                    dd = cand[j][0] - cand[j2][0]
                    if cov[e1] - dd >= v[e1] and cov[e2] + dd >= v[e2]:
                        cand[j][1][a] = e2
                        cand[j2][1][b] = e1
                        ok = True
        elif op < 0.85:
            j = rng.randrange(len(cand))
            if cand[j][1]:
                si = rng.randrange(len(cand[j][1]))
                e = cand[j][1][si]
                if cov[e] - cand[j][0] >= v[e]:
                    cand[j][1].pop(si)
                    ok = True
        else:
            e = rng.randrange(len(v))
            if v[e] > 0 and cand:
                j = rng.randrange(len(cand))
                if len(cand[j][1]) < N_CORES:
                    cand[j][1].append(e)
                    ok = True
        if not ok:
            continue
        _tighten(cand, v)
        new_c = sum(c[0] for c in cand)
        if new_c <= cur_c or rng.random() < np.exp((cur_c - new_c) / temp):
            cur = cand
            cur_c = new_c
            if new_c < best_c:
                best_c = new_c
                best = copy.deepcopy(cand)
    return best


def _schedule(v_key):
    """v (tuple of 64 ints) -> (unit widths desc, assign[j][c] = (e, c0, cov)
    or None). All cores share the width sequence."""
    if v_key in _SCHEDULE_CACHE:
        return _SCHEDULE_CACHE[v_key]
    v = [int(x) for x in v_key]
    cols = _slot_anneal(v)
    cols.sort(key=lambda c: -c[0])
    widths = tuple(c[0] for c in cols)
    nunits = len(widths)
    # expert -> list of (unit width, unit idx, core)
    slots = {}
    for j, (w, sl) in enumerate(cols):
        for c, e in enumerate(sl):
            slots.setdefault(e, []).append((w, j, c))
    assign = [[None] * N_CORES for _ in range(nunits)]
    for e, sl in slots.items():
        sl.sort(key=lambda s: -s[0])
        cum = 0
        for wu, j, c in sl:
            if cum >= v[e]:
                continue  # surplus slot -> dummy
            c0 = min(cum, CAP - wu)
            end = min(c0 + wu, v[e])
            assign[j][c] = (e, c0, end - c0)
            cum = end
        assert cum >= v[e], (e, v[e], sl)
    _SCHEDULE_CACHE[v_key] = (widths, assign)
    return widths, assign


# ----------------------------- device program -----------------------------
# All DRAM operands are pre-packed on the host into partition-major layouts
# so every DMA descriptor is 128 contiguous lines: trigger instructions cost
# engine time proportional to line count (HW-measured ~0.7us per 512-line
# trigger, 7.6us for a 2048-line w2) and a slow trigger on the ACT engine
# delays the activations that recycle PSUM banks.

def _build_program(widths: tuple):
    nc = bacc.Bacc(None, target_bir_lowering=False)
    K = len(widths)

    xt = nc.dram_tensor("xt", [K, 128, KT1 * WMAX], BF16, kind="ExternalInput")
    w1g = nc.dram_tensor("w1g", [K, 128, KT1, H], BF16, kind="ExternalInput")
    w2g = nc.dram_tensor("w2g", [K, 128, KT2, D], BF16, kind="ExternalInput")
    bg = nc.dram_tensor("bg", [K, 128, MT1 + MT2], F32, kind="ExternalInput")
    yt = nc.dram_tensor("yt", [K, 128, MT2 * WMAX], F32, kind="ExternalOutput")
    scr = nc.dram_tensor("scr", [128, 64], F32, kind="ExternalOutput")

    Gelu = mybir.ActivationFunctionType.Gelu
    Ident = mybir.ActivationFunctionType.Identity

    # 2nd-smallest unit first: its x arrives fast AND its ~14us of compute
    # covers the constant ~4.2MB/unit weight-DMA startup debt (a smaller
    # first unit starves the PE and re-throttles HAM); smallest unit last so
    # the final y writeback drain is short
    if K >= 3:
        emit_order = [K - 2] + list(range(K - 2)) + [K - 1]
    else:
        emit_order = list(range(K - 1, -1, -1))

    with tile.TileContext(nc) as tc:
        with (
            tc.tile_pool(name="wu", bufs=1) as wu,
            tc.tile_pool(name="w1p", bufs=2) as w1p,
            tc.tile_pool(name="w2p", bufs=2) as w2p,
            tc.tile_pool(name="bp", bufs=2) as bp,
            tc.tile_pool(name="xp", bufs=3) as xp,
            tc.tile_pool(name="hp", bufs=2) as hp,
            tc.tile_pool(name="yp", bufs=2) as yp,
            tc.tile_pool(name="ps_h", bufs=4, space="PSUM") as ps_h,
            tc.tile_pool(name="ps_y", bufs=4, space="PSUM") as ps_y,
        ):
            # PE pre-warm: ~5us of dummy matmuls during the initial DMA wait
            # flips the HAM clock gate to 8/8 before real work arrives
            warm = wu.tile([128, 64], BF16, tag="warm")
            nc.gpsimd.memset(warm, 0.0)
            wps = None
            for _ in range(96):
                wps = ps_h.tile([128, 64], F32, tag="psh")
                nc.tensor.matmul(
                    wps[:64, :], lhsT=warm, rhs=warm, start=True, stop=True)
            wout = wu.tile([128, 64], F32, tag="wout")
            nc.scalar.activation(wout[:64, :], wps[:64, :], Ident)
            nc.gpsimd.dma_start(out=scr[:64, :], in_=wout[:64, :])

            for ei, j in enumerate(emit_order):
                W = widths[j]
                w1_t = w1p.tile([128, KT1, H], BF16, tag="w1")
                x_t = xp.tile([128, KT1 * WMAX], BF16, tag="x")
                nc.sync.dma_start(out=x_t[:, :KT1 * W], in_=xt[j][:, :KT1 * W])
                if ei < 2:
                    # k-sliced early loads: GEMM1 starts after slice 0 + x
                    # (packed layout makes extra triggers cheap: ~0.2us each)
                    for k in range(KT1):
                        nc.sync.dma_start(
                            out=w1_t[:, k, :], in_=w1g[j][:, k, :])
                else:
                    nc.sync.dma_start(out=w1_t, in_=w1g[j])
                b_t = bp.tile([128, MT1 + MT2], F32, tag="b")
                nc.scalar.dma_start(out=b_t, in_=bg[j])
                # w2 rides the second HWDGE ring (ACT): needed only for GEMM2
                w2_t = w2p.tile([128, KT2, D], BF16, tag="w2")
                if ei >= 2:
                    nc.scalar.dma_start(out=w2_t, in_=w2g[j])

                h_t = hp.tile([128, KT2, WMAX], BF16, tag="h")
                for m in range(MT1):
                    ps = ps_h.tile([128, WMAX], F32, tag="psh")
                    for k in range(KT1):
                        nc.tensor.matmul(
                            ps[:, :W],
                            lhsT=w1_t[:, k, m * 128:(m + 1) * 128],
                            rhs=x_t[:, k * W:(k + 1) * W],
                            start=(k == 0),
                            stop=(k == KT1 - 1),
                        )
                    nc.scalar.activation(
                        h_t[:, m, :W], ps[:, :W], Gelu, bias=b_t[:, m:m + 1])
                if ei < 2:
                    # early units: w2 deferred past GEMM1 (frees startup HBM
                    # bandwidth for the critical w1/x path) and k-group
                    # sliced so GEMM2 unblocks progressively
                    for a in range(4):
                        nc.scalar.dma_start(
                            out=w2_t[:, 4 * a:4 * a + 4, :],
                            in_=w2g[j][:, 4 * a:4 * a + 4, :])

                y_t = yp.tile([128, MT2 * WMAX], F32, tag="y")
                last = ei == K - 1
                for dm in range(MT2):
                    ps2 = ps_y.tile([128, WMAX], F32, tag="psy")
                    for k in range(KT2):
                        nc.tensor.matmul(
                            ps2[:, :W],
                            lhsT=w2_t[:, k, dm * 128:(dm + 1) * 128],
                            rhs=h_t[:, k, :W],
                            start=(k == 0),
                            stop=(k == KT2 - 1),
                        )
                    nc.scalar.activation(
                        y_t[:, dm * W:(dm + 1) * W], ps2[:, :W], Ident,
                        bias=b_t[:, MT1 + dm:MT1 + dm + 1])
                    if last:
                        # final unit: write out per m-tile to shrink the tail
                        nc.gpsimd.dma_start(
                            out=yt[j][:, dm * W:(dm + 1) * W],
                            in_=y_t[:, dm * W:(dm + 1) * W])
                if not last:
                    nc.gpsimd.dma_start(
                        out=yt[j][:, :MT2 * W], in_=y_t[:, :MT2 * W])

    nc.compile()
    return nc


# ----------------------------- host wrapper -----------------------------

def kernel(packed_inputs, valid_load, w1, b1, w2, b2, _trace=False, **_):
    global LAST_RESULT
    packed_inputs = np.asarray(packed_inputs, np.float32)
    w1 = np.asarray(w1, np.float32)
    b1 = np.asarray(b1, np.float32)
    w2 = np.asarray(w2, np.float32)
    b2 = np.asarray(b2, np.float32)
    v = np.asarray(valid_load).astype(np.int64)

    out = np.zeros((E, CAP, D), np.float32)
    if int(v.max()) <= 0:
        return out

    widths, assign = _schedule(tuple(int(x) for x in v))
    K = len(widths)

    if widths not in _PROGRAM_CACHE:
        _PROGRAM_CACHE[widths] = _build_program(widths)
    nc = _PROGRAM_CACHE[widths]

    bf16 = mybir.dt.np(BF16)
    # xT per expert, partition-major k-tiles: [E, KT1, 128, CAP]
    xt_all = np.ascontiguousarray(
        packed_inputs.transpose(0, 2, 1).reshape(E, KT1, 128, CAP)
    ).astype(bf16)
    # w1 partition-major: [E, 128, KT1, H]; w2: [E, 128, KT2, D]
    w1r = np.ascontiguousarray(
        w1.astype(bf16).reshape(E, KT1, 128, H).transpose(0, 2, 1, 3))
    w2r = np.ascontiguousarray(
        w2.astype(bf16).reshape(E, KT2, 128, D).transpose(0, 2, 1, 3))
    b1r = b1.reshape(E, MT1, 128).transpose(0, 2, 1)     # [E, 128, MT1]
    b2r = b2.reshape(E, MT2, 128).transpose(0, 2, 1)
    bgr = np.ascontiguousarray(
        np.concatenate([b1r, b2r], axis=2), np.float32)   # [E, 128, 20]

    in_maps = []
    for c in range(N_CORES):
        xtc = np.zeros((K, 128, KT1 * WMAX), bf16)
        w1c = np.zeros((K, 128, KT1, H), bf16)
        w2c = np.zeros((K, 128, KT2, D), bf16)
        bc = np.zeros((K, 128, MT1 + MT2), np.float32)
        for j, W in enumerate(widths):
            pc = assign[j][c]
            if pc is None:
                continue
            e, c0, _cov = pc
            # [KT1, 128, W] -> [128, KT1*W]
            xs = xt_all[e][:, :, c0:c0 + W]
            xtc[j, :, :KT1 * W] = xs.transpose(1, 0, 2).reshape(128, KT1 * W)
            w1c[j] = w1r[e]
            w2c[j] = w2r[e]
            bc[j] = bgr[e]
        in_maps.append({"xt": xtc, "w1g": w1c, "w2g": w2c, "bg": bc})

    res = run_bass_kernel_spmd(nc, in_maps, list(range(N_CORES)), trace=_trace)
    LAST_RESULT = res

    for c in range(N_CORES):
        ytc = res.results[c]["yt"]
        for j, W in enumerate(widths):
            pc = assign[j][c]
            if pc is None:
                continue
            e, c0, cov = pc
            # [128, MT2*W] -> [(m p), W] = yT unit
            yu = ytc[j][:, :MT2 * W].reshape(128, MT2, W)
            yu = yu.transpose(1, 0, 2).reshape(D, W)
            out[e, c0:c0 + cov, :] = yu[:, :cov].T
    return out
